# revision 25
# baseline (speedup 1.0000x reference)
"""Causal multi-head attention on 8 TRN2 NeuronCores.

Sharding: core = (batch b, head-group hg): b = core//2, hg = core%2 (6 of 12
heads each). Each core computes qkv for its heads, causal attention, and a
partial out-projection; an in-NEFF pair ReduceScatter sums the two partials
per batch and leaves half the output channels on each core.

Layouts (bf16 matmul inputs, f32 PSUM accumulation):
  xT  [C=768, N=2048]   (host-transposed x[b])
  qT,kT [384, 2048]     (head-major: local head h at rows (h%2)*64..)
  v   [2048-blocks, 6, 65] (col 64 = 1.0 -> rowsum trick)
  ST  [128 j, <=512 i] = kT_h[:,jblk].T @ qT_h[:,islice]  (K=64), ragged at
      the diagonal: block jb=4*it+m only computes i-cols [128m:512).
  PT  = exp(ST/8) (batched activations over multi-bank psum groups),
      causal-masked via affine_select on the diagonal blocks
  OT  [64+1, 512] += v[jblk,h].T @ PT  (row 64 = softmax denom)
  out = (OT[0:64] * 1/denom) -> oT_sb [384, 2048] bf16
  yT = wp.T @ oT + b_proj/2 -> [768, 2048] bf16 -> pair ReduceScatter(add)
     -> out [384, 2048] bf16 (even core: channels 0:384, odd: 384:768)
"""

import numpy as np
import ml_dtypes

B, N, C = 4, 2048, 768
H, D = 12, 64
HG = 6          # heads per core
CG = HG * D     # 384 = local head channels
NCORES = 8
NB = N // 128   # 16 j-blocks
NT = N // 512   # 4 i-tiles
CCH = C // 128  # 6 contraction chunks

_COMPILED = {}


def _build(iters=1):
    import concourse.bass as bass
    import concourse.mybir as mybir
    import concourse.tile as tile
    from concourse import bacc

    fp32 = mybir.dt.float32
    bf16 = mybir.dt.bfloat16
    Exp = mybir.ActivationFunctionType.Exp
    Ident = mybir.ActivationFunctionType.Identity

    nc = bacc.Bacc(None, target_bir_lowering=False, num_devices=NCORES)
    xT = nc.declare_dram_parameter("xT", [C, N], bf16, isOutput=False)
    wq = nc.declare_dram_parameter("wq", [C, CG], bf16, isOutput=False)
    wk = nc.declare_dram_parameter("wk", [C, CG], bf16, isOutput=False)
    wv = nc.declare_dram_parameter("wv", [C, CG], bf16, isOutput=False)
    wp = nc.declare_dram_parameter("wp", [CG, C], bf16, isOutput=False)
    bias2 = nc.declare_dram_parameter("bias2", [C, 1], fp32, isOutput=False)
    out = nc.declare_dram_parameter("out", [CG, N], bf16, isOutput=True)

    scale = float(D) ** -0.5

    with tile.TileContext(nc) as tc:
        with (
            tc.tile_pool(name="persist", bufs=1) as pp,
            tc.tile_pool(name="work", bufs=4) as wkp,
            tc.tile_pool(name="outp", bufs=3) as op,
            tc.tile_pool(name="ps", bufs=2, space="PSUM") as ps,
            tc.tile_pool(name="ps_ot", bufs=1, space="PSUM") as ps_ot,
            tc.tile_pool(name="dram", bufs=1, space="DRAM") as dram,
        ):
            # Causal mask matching the combined ragged-diagonal psum tile:
            # [0:512]=m0, [512:896]=m1, [896:1024]=gap(0), [1024:1280]=m2,
            # [1280:1408]=m3; each m region keeps f_local >= p.
            mask_sb = pp.tile([128, 1408], bf16, name="mask_sb")
            nc.vector.memset(mask_sb[:], 1.0)
            nc.vector.memset(mask_sb[:, 896:1024], 0.0)
            for moff, mw in ((0, 512), (512, 384), (1024, 256), (1280, 128)):
                nc.gpsimd.affine_select(
                    out=mask_sb[:, moff:moff + mw],
                    in_=mask_sb[:, moff:moff + mw],
                    pattern=[[1, mw]],
                    compare_op=mybir.AluOpType.is_ge,
                    fill=0.0,
                    base=0,
                    channel_multiplier=-1,
                )

            for _ in range(iters):
                # ---- load inputs to SBUF ----
                xT_sb = [pp.tile([128, N], bf16, name=f"xT{i}") for i in range(CCH)]
                wq_sb = [pp.tile([128, CG], bf16, name=f"wq{i}") for i in range(CCH)]
                wk_sb = [pp.tile([128, CG], bf16, name=f"wk{i}") for i in range(CCH)]
                wv_sb = [pp.tile([128, CG], bf16, name=f"wv{i}") for i in range(CCH)]
                wp_sb = [pp.tile([128, C], bf16, name=f"wp{i}") for i in range(3)]
                bias_sb = pp.tile([128, CCH], fp32, name="bias_sb")
                for i in range(CCH):
                    nc.sync.dma_start(xT_sb[i][:], xT[i * 128:(i + 1) * 128, :])
                    nc.scalar.dma_start(wq_sb[i][:], wq[i * 128:(i + 1) * 128, :])
                    nc.scalar.dma_start(wk_sb[i][:], wk[i * 128:(i + 1) * 128, :])
                    nc.sync.dma_start(wv_sb[i][:], wv[i * 128:(i + 1) * 128, :])
                    nc.scalar.dma_start(
                        bias_sb[:, i:i + 1], bias2[i * 128:(i + 1) * 128, :])
                for i in range(3):
                    nc.scalar.dma_start(wp_sb[i][:], wp[i * 128:(i + 1) * 128, :])

                qT_sb = [pp.tile([128, N], bf16, name=f"qT{g}") for g in range(3)]
                kT_sb = [pp.tile([128, N], bf16, name=f"kT{g}") for g in range(3)]
                v_sb = [pp.tile([128, HG, 128], bf16, name=f"v{nb}")
                        for nb in range(NB)]
                oT_sb = [pp.tile([128, N], bf16, name=f"oT{g}") for g in range(3)]
                ypart = dram.tile([C, N], bf16, name="ypart")
                yhalf = dram.tile([CG, N], bf16, name="yhalf")

                def qk_pair(g):
                    # ci-outer: each weight chunk stays loaded for 4 matmuls
                    for dst, w in ((qT_sb, wq_sb), (kT_sb, wk_sb)):
                        tA = ps.tile([128, 1536], fp32, name="st", tag="st")
                        tB = ps_ot.tile([128, 512], fp32, name="qkB", tag="ot0")
                        for ci in range(CCH):
                            first, last = ci == 0, ci == CCH - 1
                            for nt in range(3):
                                nc.tensor.matmul(
                                    tA[:, nt * 512:(nt + 1) * 512],
                                    lhsT=w[ci][:, g * 128:(g + 1) * 128],
                                    rhs=xT_sb[ci][:, nt * 512:(nt + 1) * 512],
                                    start=first, stop=last,
                                )
                            nc.tensor.matmul(
                                tB[:],
                                lhsT=w[ci][:, g * 128:(g + 1) * 128],
                                rhs=xT_sb[ci][:, 1536:2048],
                                start=first, stop=last,
                            )
                        nc.vector.tensor_copy(out=dst[g][:, 0:1536], in_=tA[:])
                        nc.vector.tensor_copy(out=dst[g][:, 1536:2048], in_=tB[:])

                def v_blocks():
                    for pair in range(NB // 2):
                        pst = ps.tile([128, 1024], fp32, name="st", tag="st")
                        for bank in range(2):
                            nb = pair * 2 + bank
                            for ci in range(CCH):
                                nc.tensor.matmul(
                                    pst[:, bank * 512:bank * 512 + CG],
                                    lhsT=xT_sb[ci][:, nb * 128:(nb + 1) * 128],
                                    rhs=wv_sb[ci][:],
                                    start=(ci == 0), stop=(ci == CCH - 1),
                                )
                        for bank in range(2):
                            nb = pair * 2 + bank
                            # cols 64:128 all-ones -> AV yields 64 denom rows
                            nc.vector.memset(v_sb[nb][:, :, 64:128], 1.0)
                            nc.vector.tensor_copy(
                                out=v_sb[nb][:, :, 0:64],
                                in_=pst[:, bank * 512:bank * 512 + CG]
                                .rearrange("p (h d) -> p h d", d=64),
                            )

                def attention(g):
                    for it in range(NT):
                        ots = [ps_ot.tile([128, 512], fp32, name=f"ot{h}",
                                          tag=f"ot{h}") for h in range(2)]
                        # chunk: (regions=[(jb, col_off, width, i_off)...],
                        #         exp_width, is_diag)
                        chunks = []
                        fulls = list(range(4 * it))
                        for k in range(0, len(fulls), 3):
                            trip = fulls[k:k + 3]
                            regions = [(jb, 512 * j, 512, 0)
                                       for j, jb in enumerate(trip)]
                            chunks.append((regions, [(0, 512 * len(trip))],
                                           False))
                        # combined ragged diagonal: m0..m3 at bank-legal offs
                        diag = [(4 * it + 0, 0, 512, 0),
                                (4 * it + 1, 512, 384, 128),
                                (4 * it + 2, 1024, 256, 256),
                                (4 * it + 3, 1280, 128, 384)]
                        chunks.append((diag, [(0, 896), (1024, 384)], True))
                        jlast = 4 * it + 3
                        for regions, exp_spans, is_diag in chunks:
                            banks = [off // 512 for (_, off, _, _) in regions]
                            sts, pts = [], []
                            for h in range(2):
                                ro = h * 64
                                st = ps.tile([128, 1536], fp32, name="st",
                                             tag="st")
                                sts.append(st)
                                for ri, (jb, off, w, io) in enumerate(regions):
                                    bank = banks[ri]
                                    nc.tensor.matmul(
                                        st[:, off:off + w],
                                        lhsT=kT_sb[g][ro:ro + 64,
                                                      jb * 128:(jb + 1) * 128],
                                        rhs=qT_sb[g][ro:ro + 64,
                                                     it * 512 + io:(it + 1) * 512],
                                        start=bank not in banks[:ri],
                                        stop=bank not in banks[ri + 1:],
                                    )
                            for h in range(2):
                                pt = wkp.tile([128, 1536], bf16, name="pt",
                                              tag="pt")
                                pts.append(pt)
                                for (eo, ew) in exp_spans:
                                    nc.scalar.activation(
                                        pt[:, eo:eo + ew], sts[h][:, eo:eo + ew],
                                        Exp, scale=scale)
                                    if is_diag:
                                        nc.vector.tensor_tensor(
                                            pt[:, eo:eo + ew],
                                            pt[:, eo:eo + ew],
                                            mask_sb[:, eo:eo + ew],
                                            mybir.AluOpType.mult,
                                        )
                            for h in range(2):
                                for (jb, off, w, io) in regions:
                                    nc.tensor.matmul(
                                        ots[h][:, io:io + w],
                                        lhsT=v_sb[jb][:, 2 * g + h, :],
                                        rhs=pts[h][:, off:off + w],
                                        start=(jb == 0), stop=(jb == jlast),
                                    )
                        for h in range(2):
                            ro = h * 64
                            # ot rows 64:128 hold 64 copies of the denominator
                            rec64 = wkp.tile([64, 512], fp32, name="rec64",
                                             tag=f"rec64{h}")
                            nc.vector.reciprocal(rec64[:], ots[h][64:128, :])
                            nc.vector.tensor_tensor(
                                oT_sb[g][ro:ro + 64, it * 512:(it + 1) * 512],
                                ots[h][0:64, :],
                                rec64[:],
                                mybir.AluOpType.mult,
                            )

                def proj(nt):
                    for gpair in range(3):  # output channel chunk pairs
                        pst = ps.tile([128, 1024], fp32, name="st", tag="st")
                        for bank in range(2):
                            go = gpair * 2 + bank
                            for ci in range(3):
                                nc.tensor.matmul(
                                    pst[:, bank * 512:(bank + 1) * 512],
                                    lhsT=wp_sb[ci][:, go * 128:(go + 1) * 128],
                                    rhs=oT_sb[ci][:, nt * 512:(nt + 1) * 512],
                                    start=(ci == 0), stop=(ci == 2),
                                )
                        for bank in range(2):
                            go = gpair * 2 + bank
                            yt = op.tile([128, 512], bf16, name="yt", tag="yt")
                            nc.scalar.activation(
                                yt[:], pst[:, bank * 512:(bank + 1) * 512],
                                Ident, bias=bias_sb[:, go:go + 1])
                            nc.sync.dma_start(
                                ypart[go * 128:(go + 1) * 128,
                                      nt * 512:(nt + 1) * 512],
                                yt[:])

                qk_pair(0)
                v_blocks()
                qk_pair(1)
                attention(0)
                qk_pair(2)
                attention(1)
                attention(2)
                for nt in range(NT):
                    proj(nt)

                nc.gpsimd.collective_compute(
                    "ReduceScatter",
                    mybir.AluOpType.add,
                    replica_groups=[[0, 1], [2, 3], [4, 5], [6, 7]],
                    ins=[ypart.opt()],
                    outs=[yhalf.opt()],
                )
                nc.gpsimd.dma_start(out[:], yhalf[:])
    nc.compile()
    return nc


def _make_in_maps(x, w_qkv, w_proj, b_proj):
    bf = ml_dtypes.bfloat16
    x = np.asarray(x, np.float32)
    w_qkv = np.asarray(w_qkv, np.float32)
    w_proj = np.asarray(w_proj, np.float32)
    b2 = (np.asarray(b_proj, np.float32) / 2.0).reshape(C, 1)
    wq_f, wk_f, wv_f = w_qkv[:, :C], w_qkv[:, C:2 * C], w_qkv[:, 2 * C:]
    in_maps = []
    for core in range(NCORES):
        b, hg = core // 2, core % 2
        cs = slice(hg * CG, (hg + 1) * CG)
        in_maps.append({
            "xT": np.ascontiguousarray(x[b].T).astype(bf),
            "wq": wq_f[:, cs].astype(bf),
            "wk": wk_f[:, cs].astype(bf),
            "wv": wv_f[:, cs].astype(bf),
            "wp": w_proj[cs, :].astype(bf),
            "bias2": b2,
        })
    return in_maps


def _get_runner(nc):
    """Build (once) a cached jitted SPMD dispatch for `nc`."""
    import jax
    import numpy as np_
    from jax.sharding import Mesh, PartitionSpec
    from jax.experimental.shard_map import shard_map
    import concourse.bass2jax as b2j
    import concourse.mybir as mybir

    b2j.install_neuronx_cc_hook()

    partition_name = (nc.partition_id_tensor.name
                      if nc.partition_id_tensor else None)
    in_names, out_names, out_avals, zero_outs = [], [], [], []
    for alloc in nc.m.functions[0].allocations:
        if not isinstance(alloc, mybir.MemoryLocationSet):
            continue
        name = alloc.memorylocations[0].name
        if alloc.kind == "ExternalInput":
            if name != partition_name:
                in_names.append(name)
        elif alloc.kind == "ExternalOutput":
            shape = tuple(alloc.tensor_shape)
            dtype = mybir.dt.np(alloc.dtype)
            out_names.append(name)
            out_avals.append(jax.core.ShapedArray(shape, dtype))
            zero_outs.append(np_.zeros(shape, dtype))
    n_params = len(in_names)
    all_in_names = list(in_names) + list(out_names)
    if partition_name is not None:
        all_in_names.append(partition_name)

    def _body(*args):
        operands = list(args)
        if partition_name is not None:
            operands.append(b2j.partition_id_tensor())
        return tuple(b2j._bass_exec_p.bind(
            *operands,
            out_avals=tuple(out_avals),
            in_names=tuple(all_in_names),
            out_names=tuple(out_names),
            lowering_input_output_aliases=(),
            sim_require_finite=True,
            sim_require_nnan=True,
            nc=nc,
        ))

    devices = jax.devices()[:NCORES]
    mesh = Mesh(np_.asarray(devices), ("core",))
    in_specs = (PartitionSpec("core"),) * (n_params + len(out_avals))
    out_specs = (PartitionSpec("core"),) * len(out_names)
    sharded = jax.jit(
        shard_map(_body, mesh=mesh, in_specs=in_specs, out_specs=out_specs,
                  check_rep=False),
        keep_unused=True,
    )

    def run(in_maps):
        per_core = [[np_.asarray(m[nm]) for nm in in_names] for m in in_maps]
        concat_in = [
            np_.concatenate([per_core[c][i] for c in range(NCORES)], axis=0)
            for i in range(n_params)
        ]
        concat_zeros = [
            np_.zeros((NCORES * z.shape[0], *z.shape[1:]), z.dtype)
            for z in zero_outs
        ]
        out_arrs = sharded(*concat_in, *concat_zeros)
        return [
            {nm: np_.asarray(out_arrs[i]).reshape(NCORES, *out_avals[i].shape)[c]
             for i, nm in enumerate(out_names)}
            for c in range(NCORES)
        ]

    run.sharded = sharded
    run.in_names = in_names
    run.out_names = out_names
    run.out_avals = out_avals
    run.zero_outs = zero_outs
    run.mesh = mesh
    return run


def kernel(x, w_qkv, w_proj, b_proj):
    if "nc" not in _COMPILED:
        _COMPILED["nc"] = _build(iters=1)
        _COMPILED["runner"] = _get_runner(_COMPILED["nc"])
    runner = _COMPILED["runner"]

    in_maps = _make_in_maps(x, w_qkv, w_proj, b_proj)
    results = runner(in_maps)
    y = np.empty((B, N, C), np.float32)
    for b in range(B):
        top = np.asarray(results[2 * b]["out"], np.float32)      # ch 0:384
        bot = np.asarray(results[2 * b + 1]["out"], np.float32)  # ch 384:768
        y[b] = np.concatenate([top, bot], axis=0).T
    return y
